# revision 1
# baseline (speedup 1.0000x reference)
"""GraphTransformer (2x PyG TransformerConv + out proj) on 8 trn2 NeuronCores.

Strategy (edge-parallel via dst-ownership):
- Host: sort nodes globally by (degree, id); rank r -> core r%8, local slot
  r//8. Every core's tile t covers the same global rank block => identical
  per-tile max degree D_t on all cores (SPMD-compatible). Edges grouped by
  dst; each dst's edges live entirely on its owner core as gather slots.
- Device, per layer: node-sharded fp32 projections on PE (q,k,v,skip),
  AllGather of fused k|v table [50176,1024]; per 128-node tile: indirect-DMA
  gather of k|v rows per slot chunk, DVE dot + exp (softmax is shift
  invariant; logits are small, so no segment max needed) + masked sum +
  weighted sum, then skip-add + ReLU; PE-transpose of h for next layer's
  stationary operand. Final projection -> per-core output shard; host
  un-permutes rows.
"""
import numpy as np

N, E, D, H, C, HC = 50000, 400000, 384, 4, 128, 512
NCORES, P = 8, 128
NLOC = N // NCORES
NTILES = (NLOC + P - 1) // P
SHARD = NTILES * P
NPAD = SHARD * NCORES
SCHUNK = 8
INV_SQRT_C = 1.0 / np.sqrt(np.float32(C))


# ---------------------------------------------------------------- host prep
def _prep(edge_index):
    src = np.asarray(edge_index[0], dtype=np.int64)
    dst = np.asarray(edge_index[1], dtype=np.int64)
    deg = np.bincount(dst, minlength=N)
    node_of_rank = np.lexsort((np.arange(N), deg))
    rank_of_node = np.empty(N, np.int64)
    rank_of_node[node_of_rank] = np.arange(N)
    g_of_rank = (np.arange(N) % NCORES) * SHARD + (np.arange(N) // NCORES)
    g_of_node = np.empty(N, np.int64)
    g_of_node[node_of_rank] = g_of_rank

    deg_sorted = deg[node_of_rank]
    Dts = []
    for t in range(NTILES):
        blk = deg_sorted[t * P * NCORES:(t + 1) * P * NCORES]
        Dts.append(max(int(blk.max()) if len(blk) else 0, 1))
    SUMD = sum(Dts)
    coloff = np.cumsum([0] + Dts)[:-1]

    er = rank_of_node[dst]
    order = np.argsort(er, kind="stable")
    er_s = er[order]
    gsrc_s = g_of_node[src[order]]
    starts = np.searchsorted(er_s, np.arange(N))
    slot = np.arange(E) - starts[er_s]

    core_e = er_s % NCORES
    local_e = er_s // NCORES
    col_e = coloff[local_e // P] + slot
    p_e = local_e % P

    srcidx = np.zeros((NCORES, P, SUMD), np.int32)
    mask = np.zeros((NCORES, P, SUMD), np.float32)
    srcidx[core_e, p_e, col_e] = gsrc_s.astype(np.int32)
    mask[core_e, p_e, col_e] = 1.0
    maskH = np.repeat(mask, H, axis=2)
    return srcidx, maskH, Dts, SUMD, coloff, node_of_rank


def _shard_rows(x, node_of_rank):
    D_in = x.shape[1]
    out = np.zeros((NCORES, SHARD, D_in), np.float32)
    r = np.arange(N)
    out[r % NCORES, r // NCORES] = x[node_of_rank]
    return out


# ---------------------------------------------------------------- wait fix
def _split_waits(nc):
    """walrus here rejects >1 sem-wait per instruction; split extras onto
    InstNoOp carriers inserted just before, same engine."""
    import concourse.mybir as mybir
    for fn in nc.m.functions:
        for bb in fn.blocks:
            out = []
            changed = False
            for ins in bb.instructions:
                si = ins.sync_info
                waits = list(si.on_wait) if si and si.on_wait else []
                if len(waits) > 1:
                    changed = True
                    for j, w in enumerate(waits[:-1]):
                        out.append(mybir.InstNoOp(
                            name=f"{ins.name}-wf{j}", opcode="NoOp",
                            engine=ins.engine,
                            sync_info=mybir.SyncInfo(on_wait=[w], on_update=[]),
                            text_hint="waitfix"))
                    si.on_wait = waits[-1:]
                out.append(ins)
            if changed:
                bb.instructions = out


# ---------------------------------------------------------------- bass build
def _build_nc(Dts, SUMD, coloff, stop_after=None):
    import concourse.bass as bass
    import concourse.mybir as mybir
    import concourse.tile as tile
    from concourse.masks import make_identity
    f32 = mybir.dt.float32

    nc = bass.Bass(num_devices=NCORES)
    xT = nc.dram_tensor("xT", [D, SHARD], f32, kind="ExternalInput")
    srcidx_d = nc.dram_tensor("srcidx", [P, SUMD], mybir.dt.int32, kind="ExternalInput")
    maskH_d = nc.dram_tensor("maskH", [P, SUMD * H], f32, kind="ExternalInput")
    wT, bia = {}, {}
    for l, Din in ((0, D), (1, HC)):
        for nm in "qkvs":
            wT[nm, l] = nc.dram_tensor(f"w{nm}{l}T", [Din, HC], f32, kind="ExternalInput")
            bia[nm, l] = nc.dram_tensor(f"b{nm}{l}", [1, HC], f32, kind="ExternalInput")
    woutT = nc.dram_tensor("woutT", [HC, D], f32, kind="ExternalInput")
    bout = nc.dram_tensor("bout", [1, D], f32, kind="ExternalInput")
    out_d = nc.dram_tensor("out", [SHARD, D], f32, kind="ExternalOutput")

    chunks = []  # per tile: list of (coloff, S)
    for t in range(NTILES):
        cs, off = [], 0
        while off < Dts[t]:
            cs.append((int(coloff[t]) + off, min(SCHUNK, Dts[t] - off)))
            off += SCHUNK
        chunks.append(cs)

    with tile.TileContext(nc) as tc:
        with (
            tc.tile_pool(name="dram", bufs=1, space="DRAM") as dram,
            tc.tile_pool(name="const", bufs=1) as const,
        ):
            # persistent DRAM scratch
            qd = [dram.tile([SHARD, HC], f32, name=f"q{l}d") for l in range(2)]
            sd = [dram.tile([SHARD, HC], f32, name=f"s{l}d") for l in range(2)]
            kvin = [dram.tile([SHARD, 2 * HC], f32, name=f"kv{l}in") for l in range(2)]
            kvfull = [dram.tile([NPAD, 2 * HC], f32, name=f"kv{l}full", addr_space="Shared")
                      for l in range(2)]
            hT = [dram.tile([HC, SHARD], f32, name=f"h{l}T") for l in range(2)]

            # constants in SBUF
            ident = const.tile([P, P], f32)
            make_identity(nc, ident[:])
            ones = const.tile([1, P], f32)
            nc.vector.memset(ones[:], 1.0)
            srcidx_s = const.tile([P, SUMD], mybir.dt.int32)
            nc.sync.dma_start(srcidx_s[:], srcidx_d[:])
            maskH_s = const.tile([P, SUMD * H], f32)
            nc.sync.dma_start(maskH_s[:], maskH_d[:])
            bias_s = {}
            for l in range(2):
                for nm in "qkvs":
                    bias_s[nm, l] = const.tile([1, HC], f32, name=f"b{nm}{l}s")
                    nc.sync.dma_start(bias_s[nm, l][:], bia[nm, l][:])
            bout_s = const.tile([1, D], f32)
            nc.sync.dma_start(bout_s[:], bout[:])

            def projections(l, lhsT_dram, Din):
                """q,k,v,s = lhsT.T @ W^T + b for this core's SHARD rows."""
                KB = Din // P
                with (
                    tc.tile_pool(name=f"wp{l}", bufs=1) as wp,
                    tc.tile_pool(name=f"lp{l}", bufs=2 * KB + 2) as lp,
                    tc.tile_pool(name=f"op{l}", bufs=2) as op,
                    tc.tile_pool(name=f"pp{l}", bufs=2, space="PSUM") as pp,
                ):
                    w_s = {}
                    for nm in "qkvs":
                        w_s[nm] = wp.tile([P, KB * HC], f32, name=f"w{nm}s")
                        nc.sync.dma_start(
                            w_s[nm][:].rearrange("p (kb n) -> p kb n", n=HC),
                            wT[nm, l][:].rearrange("(kb p) n -> p kb n", p=P))
                    # k,v first across all tiles so the AllGather can start as
                    # early as possible; q,s then overlap the collective.
                    for group in ("kv", "qs"):
                        for t in range(NTILES):
                            lhs = []
                            for kb in range(KB):
                                lt = lp.tile([P, P], f32, tag=f"lhs{group}")
                                nc.sync.dma_start(
                                    lt[:], lhsT_dram[kb * P:(kb + 1) * P, t * P:(t + 1) * P])
                                lhs.append(lt)
                            rows = slice(t * P, (t + 1) * P)
                            for nm in group:
                                ps = pp.tile([P, HC], f32, tag=f"ps{nm}")
                                for kb in range(KB):
                                    nc.tensor.matmul(
                                        ps[:], lhsT=lhs[kb][:],
                                        rhs=w_s[nm][:].rearrange("p (kb n) -> p kb n", n=HC)[:, kb, :],
                                        start=(kb == 0), stop=False)
                                nc.tensor.matmul(
                                    ps[:], lhsT=ones[:1, :], rhs=bias_s[nm, l][:1, :],
                                    start=False, stop=True)
                                o = op.tile([P, HC], f32, tag=f"o{nm}")
                                nc.vector.tensor_copy(o[:], ps[:])
                                if nm == "q":
                                    nc.sync.dma_start(qd[l][rows, :], o[:])
                                elif nm == "s":
                                    nc.sync.dma_start(sd[l][rows, :], o[:])
                                elif nm == "k":
                                    nc.sync.dma_start(kvin[l][rows, 0:HC], o[:])
                                else:
                                    nc.sync.dma_start(kvin[l][rows, HC:2 * HC], o[:])

            def edge_phase(l):
                with (
                    tc.tile_pool(name=f"ek{l}", bufs=3) as ek,
                    tc.tile_pool(name=f"eg{l}", bufs=2) as eg,
                    tc.tile_pool(name=f"eh{l}", bufs=2) as eh,
                    tc.tile_pool(name=f"et{l}", bufs=4, space="PSUM") as et,
                ):
                    for t in range(NTILES):
                        rows = slice(t * P, (t + 1) * P)
                        q_t = eg.tile([P, HC], f32, tag="q")
                        nc.sync.dma_start(q_t[:], qd[l][rows, :])
                        s_t = eg.tile([P, HC], f32, tag="s")
                        nc.sync.dma_start(s_t[:], sd[l][rows, :])
                        den = eh.tile([P, H], f32, tag="den")
                        msg = eh.tile([P, HC], f32, tag="msg")
                        for ci, (co, S) in enumerate(chunks[t]):
                            kvg = ek.tile([P, SCHUNK * 2 * HC], f32, tag="kvg")
                            for s in range(S):
                                nc.gpsimd.indirect_dma_start(
                                    out=kvg[:, s * 2 * HC:(s + 1) * 2 * HC],
                                    out_offset=None,
                                    in_=kvfull[l][:],
                                    in_offset=bass.IndirectOffsetOnAxis(
                                        ap=srcidx_s[:, co + s:co + s + 1], axis=0))
                            kv3 = kvg[:].rearrange("p (s kv) -> p s kv", kv=2 * HC)
                            prod = eg.tile([P, SCHUNK * HC], f32, tag="prod")
                            nc.vector.tensor_tensor(
                                out=prod[:].rearrange("p (s n) -> p s n", n=HC)[:, :S],
                                in0=kv3[:, :S, 0:HC],
                                in1=q_t[:, None, :].to_broadcast([P, S, HC]),
                                op=mybir.AluOpType.mult)
                            alpha = eh.tile([P, SCHUNK * H], f32, tag="alpha")
                            nc.vector.tensor_reduce(
                                alpha[:, :S * H],
                                prod[:, :S * HC].rearrange("p (sh c) -> p sh c", c=C),
                                axis=mybir.AxisListType.X, op=mybir.AluOpType.add)
                            e_t = eh.tile([P, SCHUNK * H], f32, tag="e")
                            nc.scalar.activation(
                                e_t[:, :S * H], alpha[:, :S * H],
                                mybir.ActivationFunctionType.Exp, scale=float(INV_SQRT_C))
                            nc.vector.tensor_tensor(
                                out=e_t[:, :S * H], in0=e_t[:, :S * H],
                                in1=maskH_s[:, co * H:(co + S) * H],
                                op=mybir.AluOpType.mult)
                            if ci == 0:
                                nc.vector.tensor_reduce(
                                    den[:], e_t[:, :S * H].rearrange("p (s h) -> p h s", h=H),
                                    axis=mybir.AxisListType.X, op=mybir.AluOpType.add)
                            else:
                                den_c = eh.tile([P, H], f32, tag="denc")
                                nc.vector.tensor_reduce(
                                    den_c[:], e_t[:, :S * H].rearrange("p (s h) -> p h s", h=H),
                                    axis=mybir.AxisListType.X, op=mybir.AluOpType.add)
                                nc.vector.tensor_add(den[:], den[:], den_c[:])
                            ev = eg.tile([P, SCHUNK * HC], f32, tag="prod")
                            nc.vector.tensor_tensor(
                                out=ev[:].rearrange("p (s h c) -> p s h c", h=H, c=C)[:, :S],
                                in0=kv3[:, :S, HC:2 * HC].rearrange("p s (h c) -> p s h c", c=C),
                                in1=e_t[:, :S * H].rearrange("p (s h) -> p s h", h=H)
                                    [:, :, :, None].to_broadcast([P, S, H, C]),
                                op=mybir.AluOpType.mult)
                            if ci == 0:
                                nc.vector.tensor_reduce(
                                    msg[:], ev[:, :S * HC].rearrange("p (s n) -> p n s", n=HC),
                                    axis=mybir.AxisListType.X, op=mybir.AluOpType.add)
                            else:
                                msg_c = eh.tile([P, HC], f32, tag="msgc")
                                nc.vector.tensor_reduce(
                                    msg_c[:], ev[:, :S * HC].rearrange("p (s n) -> p n s", n=HC),
                                    axis=mybir.AxisListType.X, op=mybir.AluOpType.add)
                                nc.vector.tensor_add(msg[:], msg[:], msg_c[:])
                        nc.scalar.activation(den[:], den[:],
                                             mybir.ActivationFunctionType.Copy, bias=1e-16)
                        rden = eh.tile([P, H], f32, tag="rden")
                        nc.vector.reciprocal(rden[:], den[:])
                        h_t = eh.tile([P, HC], f32, tag="h")
                        nc.vector.tensor_tensor(
                            out=h_t[:].rearrange("p (h c) -> p h c", c=C),
                            in0=msg[:].rearrange("p (h c) -> p h c", c=C),
                            in1=rden[:, :, None].to_broadcast([P, H, C]),
                            op=mybir.AluOpType.mult)
                        nc.vector.tensor_add(h_t[:], h_t[:], s_t[:])
                        nc.scalar.activation(h_t[:], h_t[:], mybir.ActivationFunctionType.Relu)
                        for kb in range(HC // P):
                            tp = et.tile([P, P], f32, tag="tp")
                            nc.tensor.transpose(tp[:], h_t[:, kb * P:(kb + 1) * P], ident[:])
                            o = eh.tile([P, P], f32, tag="tpo")
                            nc.scalar.copy(o[:], tp[:])
                            nc.sync.dma_start(hT[l][kb * P:(kb + 1) * P, t * P:(t + 1) * P], o[:])

            rg = [list(range(NCORES))]

            def dummy_out():
                with tc.tile_pool(name="dummy", bufs=1) as dp:
                    z = dp.tile([P, D], f32)
                    nc.vector.memset(z[:], 0.0)
                    for t in range(NTILES):
                        nc.sync.dma_start(out_d[t * P:(t + 1) * P, :], z[:])

            steps = ["p1", "ag1", "e1", "p2", "ag2", "e2", None]
            upto = steps.index(stop_after) if stop_after else len(steps) - 1

            if upto >= 0:
                projections(0, xT[:], D)
            if upto >= 1:
                nc.gpsimd.collective_compute(
                    "AllGather", mybir.AluOpType.bypass, replica_groups=rg,
                    ins=[kvin[0][:].opt()], outs=[kvfull[0][:].opt()])
            if upto >= 2:
                edge_phase(0)
            if upto >= 3:
                projections(1, hT[0][:], HC)
            if upto >= 4:
                nc.gpsimd.collective_compute(
                    "AllGather", mybir.AluOpType.bypass, replica_groups=rg,
                    ins=[kvin[1][:].opt()], outs=[kvfull[1][:].opt()])
            if upto >= 5:
                edge_phase(1)
            if stop_after is not None:
                dummy_out()

            # final projection: out = h2 @ woutT + bout
            from contextlib import ExitStack
            with ExitStack() as _st:
              if stop_after is None:
                fw = _st.enter_context(tc.tile_pool(name="fw", bufs=1))
                fl = _st.enter_context(tc.tile_pool(name="fl", bufs=10))
                fo = _st.enter_context(tc.tile_pool(name="fo", bufs=2))
                fp = _st.enter_context(tc.tile_pool(name="fp", bufs=2, space="PSUM"))
                KB = HC // P
                wo_s = fw.tile([P, KB * D], f32)
                nc.sync.dma_start(
                    wo_s[:].rearrange("p (kb n) -> p kb n", n=D),
                    woutT[:].rearrange("(kb p) n -> p kb n", p=P))
                for t in range(NTILES):
                    ps = fp.tile([P, D], f32, tag="po")
                    for kb in range(KB):
                        lt = fl.tile([P, P], f32, tag="flhs")
                        nc.sync.dma_start(
                            lt[:], hT[1][kb * P:(kb + 1) * P, t * P:(t + 1) * P])
                        nc.tensor.matmul(
                            ps[:], lhsT=lt[:],
                            rhs=wo_s[:].rearrange("p (kb n) -> p kb n", n=D)[:, kb, :],
                            start=(kb == 0), stop=False)
                    nc.tensor.matmul(ps[:], lhsT=ones[:1, :], rhs=bout_s[:1, :],
                                     start=False, stop=True)
                    o = fo.tile([P, D], f32, tag="oo")
                    nc.vector.tensor_copy(o[:], ps[:])
                    nc.sync.dma_start(out_d[t * P:(t + 1) * P, :], o[:])

    _split_waits(nc)
    return nc


def _make_in_maps(inputs, srcidx, maskH, node_of_rank):
    x = np.ascontiguousarray(np.asarray(inputs["x"], np.float32))
    xsh = _shard_rows(x, node_of_rank)
    g = lambda n: np.ascontiguousarray(np.asarray(inputs[n], np.float32))
    common = {}
    for l in range(2):
        for nm in "qkvs":
            common[f"w{nm}{l}T"] = np.ascontiguousarray(g(f"{nm}{l}_w").T)
            common[f"b{nm}{l}"] = g(f"{nm}{l}_b").reshape(1, HC)
    common["woutT"] = np.ascontiguousarray(g("out_w").T)
    common["bout"] = g("out_b").reshape(1, D)
    in_maps = []
    for c in range(NCORES):
        m = dict(common)
        m["xT"] = np.ascontiguousarray(xsh[c].T)
        m["srcidx"] = np.ascontiguousarray(srcidx[c])
        m["maskH"] = np.ascontiguousarray(maskH[c])
        in_maps.append(m)
    return in_maps


def kernel(**inputs):
    from concourse.bass_utils import run_bass_kernel_spmd
    srcidx, maskH, Dts, SUMD, coloff, node_of_rank = _prep(np.asarray(inputs["edge_index"]))
    nc = _build_nc(Dts, SUMD, coloff)
    in_maps = _make_in_maps(inputs, srcidx, maskH, node_of_rank)
    res = run_bass_kernel_spmd(nc, in_maps, core_ids=list(range(NCORES)))
    shards = np.stack([res.results[c]["out"] for c in range(NCORES)])
    full = np.empty((N, D), np.float32)
    r = np.arange(N)
    full[node_of_rank] = shards[r % NCORES, r // NCORES]
    return full



# revision 3
# speedup vs baseline: 1.6412x; 1.6412x over previous
"""GraphTransformer (2x PyG TransformerConv + out proj) on 8 trn2 NeuronCores.

Strategy (edge-parallel via dst-ownership):
- Host: sort nodes globally by (degree, id); rank r -> core r%8, local slot
  r//8. Every core's tile t covers the same global rank block => identical
  per-tile max degree D_t on all cores (SPMD-compatible). Edges grouped by
  dst; each dst's edges live entirely on its owner core as gather slots.
- Device, per layer: node-sharded bf16 projections on PE (q,k,v,skip),
  AllGather of fused k|v table [51200,1024] bf16; per 128-node tile:
  indirect-DMA gather of k|v rows per slot chunk, DVE dot + exp (softmax is
  shift invariant; logits are small, so no segment max needed). Invalid
  slots gather a zeroed padding row => contribute exp(0)=1 to the denom,
  corrected by subtracting the host-known invalid count. Weighted sums in
  fp32, then skip-add + ReLU; PE-transpose of h for next layer's stationary
  operand. Final projection -> per-core fp32 output shard; host un-permutes.
"""
import numpy as np
import ml_dtypes

N, E, D, H, C, HC = 50000, 400000, 384, 4, 128, 512
NCORES, P = 8, 128
NLOC = N // NCORES
NTILES = (NLOC + P - 1) // P
SHARD = NTILES * P
NPAD = SHARD * NCORES
SCHUNK = 8
INV_SQRT_C = 1.0 / np.sqrt(np.float32(C))
BF16 = ml_dtypes.bfloat16


# ---------------------------------------------------------------- host prep
def _prep(edge_index):
    src = np.asarray(edge_index[0], dtype=np.int64)
    dst = np.asarray(edge_index[1], dtype=np.int64)
    deg = np.bincount(dst, minlength=N)
    node_of_rank = np.lexsort((np.arange(N), deg))
    rank_of_node = np.empty(N, np.int64)
    rank_of_node[node_of_rank] = np.arange(N)
    g_of_rank = (np.arange(N) % NCORES) * SHARD + (np.arange(N) // NCORES)
    g_of_node = np.empty(N, np.int64)
    g_of_node[node_of_rank] = g_of_rank

    deg_sorted = deg[node_of_rank]
    Dts = []
    for t in range(NTILES):
        blk = deg_sorted[t * P * NCORES:(t + 1) * P * NCORES]
        Dts.append(max(int(blk.max()) if len(blk) else 0, 1))
    SUMD = sum(Dts)
    coloff = np.cumsum([0] + Dts)[:-1]

    er = rank_of_node[dst]
    order = np.argsort(er, kind="stable")
    er_s = er[order]
    gsrc_s = g_of_node[src[order]]
    starts = np.searchsorted(er_s, np.arange(N))
    slot = np.arange(E) - starts[er_s]

    core_e = er_s % NCORES
    local_e = er_s // NCORES
    col_e = coloff[local_e // P] + slot
    p_e = local_e % P

    # invalid slots gather the (zeroed) padding row SHARD-1 of core 0
    srcidx = np.full((NCORES, P, SUMD), SHARD - 1, np.int32)
    valid = np.zeros((NCORES, P, SUMD), np.float32)
    srcidx[core_e, p_e, col_e] = gsrc_s.astype(np.int32)
    valid[core_e, p_e, col_e] = 1.0
    # per-(core, p, tile) count of invalid slots, for denom correction
    ninv = np.zeros((NCORES, P, NTILES), np.float32)
    for t in range(NTILES):
        a, b = int(coloff[t]), int(coloff[t]) + Dts[t]
        ninv[:, :, t] = Dts[t] - valid[:, :, a:b].sum(axis=2)
    return srcidx, ninv, Dts, SUMD, coloff, node_of_rank


def _shard_rows(x, node_of_rank):
    D_in = x.shape[1]
    out = np.zeros((NCORES, SHARD, D_in), np.float32)
    r = np.arange(N)
    out[r % NCORES, r // NCORES] = x[node_of_rank]
    return out


# ---------------------------------------------------------------- wait fix
def _split_waits(nc):
    """walrus here rejects >1 sem-wait per instruction; split extras onto
    InstNoOp carriers inserted just before, same engine."""
    import concourse.mybir as mybir
    for fn in nc.m.functions:
        for bb in fn.blocks:
            out = []
            changed = False
            for ins in bb.instructions:
                si = ins.sync_info
                waits = list(si.on_wait) if si and si.on_wait else []
                if len(waits) > 1:
                    changed = True
                    for j, w in enumerate(waits[:-1]):
                        out.append(mybir.InstNoOp(
                            name=f"{ins.name}-wf{j}", opcode="NoOp",
                            engine=ins.engine,
                            sync_info=mybir.SyncInfo(on_wait=[w], on_update=[]),
                            text_hint="waitfix"))
                    si.on_wait = waits[-1:]
                out.append(ins)
            if changed:
                bb.instructions = out


# ---------------------------------------------------------------- bass build
def _build_nc(Dts, SUMD, coloff, stop_after=None):
    import concourse.bass as bass
    import concourse.mybir as mybir
    import concourse.tile as tile
    from concourse.masks import make_identity
    f32 = mybir.dt.float32
    bf16 = mybir.dt.bfloat16

    nc = bass.Bass(num_devices=NCORES)
    xT = nc.dram_tensor("xT", [D, SHARD], bf16, kind="ExternalInput")
    srcidx_d = nc.dram_tensor("srcidx", [P, SUMD], mybir.dt.int32, kind="ExternalInput")
    ninv_d = nc.dram_tensor("ninv", [P, NTILES], f32, kind="ExternalInput")
    wT, bia = {}, {}
    for l, Din in ((0, D), (1, HC)):
        for nm in "qkvs":
            wT[nm, l] = nc.dram_tensor(f"w{nm}{l}T", [Din, HC], bf16, kind="ExternalInput")
            bia[nm, l] = nc.dram_tensor(f"b{nm}{l}", [1, HC], bf16, kind="ExternalInput")
    woutT = nc.dram_tensor("woutT", [HC, D], bf16, kind="ExternalInput")
    bout = nc.dram_tensor("bout", [1, D], bf16, kind="ExternalInput")
    out_d = nc.dram_tensor("out", [SHARD, D], f32, kind="ExternalOutput")

    chunks = []  # per tile: list of (coloff, S)
    for t in range(NTILES):
        cs, off = [], 0
        while off < Dts[t]:
            cs.append((int(coloff[t]) + off, min(SCHUNK, Dts[t] - off)))
            off += SCHUNK
        chunks.append(cs)

    with tile.TileContext(nc) as tc:
        with (
            tc.tile_pool(name="dram", bufs=1, space="DRAM") as dram,
            tc.tile_pool(name="const", bufs=1) as const,
        ):
            # persistent DRAM scratch
            qd = [dram.tile([SHARD, HC], bf16, name=f"q{l}d") for l in range(2)]
            sd = [dram.tile([SHARD, HC], bf16, name=f"s{l}d") for l in range(2)]
            kvin = [dram.tile([SHARD, 2 * HC], bf16, name=f"kv{l}in") for l in range(2)]
            kvfull = [dram.tile([NPAD, 2 * HC], bf16, name=f"kv{l}full", addr_space="Shared")
                      for l in range(2)]
            hT = [dram.tile([HC, SHARD], bf16, name=f"h{l}T") for l in range(2)]

            # constants in SBUF
            ident = const.tile([P, P], bf16)
            make_identity(nc, ident[:])
            ones = const.tile([1, P], bf16)
            nc.vector.memset(ones[:], 1.0)
            srcidx_s = const.tile([P, SUMD], mybir.dt.int32)
            nc.sync.dma_start(srcidx_s[:], srcidx_d[:])
            ninv_s = const.tile([P, NTILES], f32)
            nc.sync.dma_start(ninv_s[:], ninv_d[:])
            bias_s = {}
            for l in range(2):
                for nm in "qkvs":
                    bias_s[nm, l] = const.tile([1, HC], bf16, name=f"b{nm}{l}s")
                    nc.sync.dma_start(bias_s[nm, l][:], bia[nm, l][:])
            bout_s = const.tile([1, D], bf16)
            nc.sync.dma_start(bout_s[:], bout[:])

            def projections(l, lhsT_dram, Din):
                """q,k,v,s = lhsT.T @ W^T + b for this core's SHARD rows."""
                KB = Din // P
                with (
                    tc.tile_pool(name=f"wp{l}", bufs=1) as wp,
                    tc.tile_pool(name=f"lp{l}", bufs=2 * KB + 2) as lp,
                    tc.tile_pool(name=f"op{l}", bufs=2) as op,
                    tc.tile_pool(name=f"pp{l}", bufs=2, space="PSUM") as pp,
                ):
                    w_s = {}
                    for nm in "qkvs":
                        w_s[nm] = wp.tile([P, KB * HC], bf16, name=f"w{nm}s")
                        nc.sync.dma_start(
                            w_s[nm][:].rearrange("p (kb n) -> p kb n", n=HC),
                            wT[nm, l][:].rearrange("(kb p) n -> p kb n", p=P))
                    # k,v first across all tiles so the AllGather can start as
                    # early as possible; q,s then overlap the collective.
                    for group in ("kv", "qs"):
                        for t in range(NTILES):
                            lhs = []
                            for kb in range(KB):
                                lt = lp.tile([P, P], bf16, tag=f"lhs{group}")
                                nc.sync.dma_start(
                                    lt[:], lhsT_dram[kb * P:(kb + 1) * P, t * P:(t + 1) * P])
                                lhs.append(lt)
                            rows = slice(t * P, (t + 1) * P)
                            for nm in group:
                                ps = pp.tile([P, HC], f32, tag=f"ps{nm}")
                                for kb in range(KB):
                                    nc.tensor.matmul(
                                        ps[:], lhsT=lhs[kb][:],
                                        rhs=w_s[nm][:].rearrange("p (kb n) -> p kb n", n=HC)[:, kb, :],
                                        start=(kb == 0), stop=False)
                                nc.tensor.matmul(
                                    ps[:], lhsT=ones[:1, :], rhs=bias_s[nm, l][:1, :],
                                    start=False, stop=True)
                                o = op.tile([P, HC], bf16, tag=f"o{nm}")
                                nc.vector.tensor_copy(o[:], ps[:])
                                if nm == "q":
                                    nc.sync.dma_start(qd[l][rows, :], o[:])
                                elif nm == "s":
                                    nc.sync.dma_start(sd[l][rows, :], o[:])
                                elif nm == "k":
                                    nc.sync.dma_start(kvin[l][rows, 0:HC], o[:])
                                else:
                                    nc.sync.dma_start(kvin[l][rows, HC:2 * HC], o[:])
                    # zero the padding row so invalid slots gather k=v=0
                    zpad = op.tile([1, 2 * HC], bf16, tag="zpad")
                    nc.vector.memset(zpad[:], 0.0)
                    nc.sync.dma_start(kvin[l][SHARD - 1:SHARD, :], zpad[:])

            def edge_phase(l):
                with (
                    tc.tile_pool(name=f"ek{l}", bufs=3) as ek,
                    tc.tile_pool(name=f"eg{l}", bufs=2) as eg,
                    tc.tile_pool(name=f"eh{l}", bufs=2) as eh,
                    tc.tile_pool(name=f"et{l}", bufs=4, space="PSUM") as et,
                ):
                    for t in range(NTILES):
                        rows = slice(t * P, (t + 1) * P)
                        q_t = eg.tile([P, HC], bf16, tag="q")
                        nc.sync.dma_start(q_t[:], qd[l][rows, :])
                        s_t = eg.tile([P, HC], bf16, tag="s")
                        nc.sync.dma_start(s_t[:], sd[l][rows, :])
                        den = eh.tile([P, H], f32, tag="den")
                        msg = eh.tile([P, HC], f32, tag="msg")
                        for ci, (co, S) in enumerate(chunks[t]):
                            kvg = ek.tile([P, SCHUNK * 2 * HC], bf16, tag="kvg")
                            for s in range(S):
                                nc.gpsimd.indirect_dma_start(
                                    out=kvg[:, s * 2 * HC:(s + 1) * 2 * HC],
                                    out_offset=None,
                                    in_=kvfull[l][:],
                                    in_offset=bass.IndirectOffsetOnAxis(
                                        ap=srcidx_s[:, co + s:co + s + 1], axis=0))
                            kv3 = kvg[:].rearrange("p (s kv) -> p s kv", kv=2 * HC)
                            prod = eg.tile([P, SCHUNK * HC], bf16, tag="prod")
                            nc.vector.tensor_tensor(
                                out=prod[:].rearrange("p (s n) -> p s n", n=HC)[:, :S],
                                in0=kv3[:, :S, 0:HC],
                                in1=q_t[:, None, :].to_broadcast([P, S, HC]),
                                op=mybir.AluOpType.mult)
                            alpha = eh.tile([P, SCHUNK * H], f32, tag="alpha")
                            nc.vector.tensor_reduce(
                                alpha[:, :S * H],
                                prod[:, :S * HC].rearrange("p (sh c) -> p sh c", c=C),
                                axis=mybir.AxisListType.X, op=mybir.AluOpType.add)
                            e_t = eh.tile([P, SCHUNK * H], bf16, tag="e")
                            nc.scalar.activation(
                                e_t[:, :S * H], alpha[:, :S * H],
                                mybir.ActivationFunctionType.Exp, scale=float(INV_SQRT_C))
                            if ci == 0:
                                nc.vector.tensor_reduce(
                                    den[:], e_t[:, :S * H].rearrange("p (s h) -> p h s", h=H),
                                    axis=mybir.AxisListType.X, op=mybir.AluOpType.add)
                            else:
                                den_c = eh.tile([P, H], f32, tag="denc")
                                nc.vector.tensor_reduce(
                                    den_c[:], e_t[:, :S * H].rearrange("p (s h) -> p h s", h=H),
                                    axis=mybir.AxisListType.X, op=mybir.AluOpType.add)
                                nc.vector.tensor_add(den[:], den[:], den_c[:])
                            ev = eg.tile([P, SCHUNK * HC], bf16, tag="prod")
                            nc.vector.tensor_tensor(
                                out=ev[:].rearrange("p (s h c) -> p s h c", h=H, c=C)[:, :S],
                                in0=kv3[:, :S, HC:2 * HC].rearrange("p s (h c) -> p s h c", c=C),
                                in1=e_t[:, :S * H].rearrange("p (s h) -> p s h", h=H)
                                    [:, :, :, None].to_broadcast([P, S, H, C]),
                                op=mybir.AluOpType.mult)
                            if ci == 0:
                                nc.vector.tensor_reduce(
                                    msg[:], ev[:, :S * HC].rearrange("p (s n) -> p n s", n=HC),
                                    axis=mybir.AxisListType.X, op=mybir.AluOpType.add)
                            else:
                                msg_c = eh.tile([P, HC], f32, tag="msgc")
                                nc.vector.tensor_reduce(
                                    msg_c[:], ev[:, :S * HC].rearrange("p (s n) -> p n s", n=HC),
                                    axis=mybir.AxisListType.X, op=mybir.AluOpType.add)
                                nc.vector.tensor_add(msg[:], msg[:], msg_c[:])
                        # subtract invalid-slot contribution (exp(0)=1 each)
                        nc.vector.tensor_tensor(
                            out=den[:], in0=den[:],
                            in1=ninv_s[:, t:t + 1].to_broadcast([P, H]),
                            op=mybir.AluOpType.subtract)
                        nc.scalar.activation(den[:], den[:],
                                             mybir.ActivationFunctionType.Copy, bias=1e-16)
                        rden = eh.tile([P, H], f32, tag="rden")
                        nc.vector.reciprocal(rden[:], den[:])
                        h_t = eh.tile([P, HC], bf16, tag="h")
                        nc.vector.tensor_tensor(
                            out=h_t[:].rearrange("p (h c) -> p h c", c=C),
                            in0=msg[:].rearrange("p (h c) -> p h c", c=C),
                            in1=rden[:, :, None].to_broadcast([P, H, C]),
                            op=mybir.AluOpType.mult)
                        nc.vector.tensor_add(h_t[:], h_t[:], s_t[:])
                        nc.scalar.activation(h_t[:], h_t[:], mybir.ActivationFunctionType.Relu)
                        for kb in range(HC // P):
                            tp = et.tile([P, P], bf16, tag="tp")
                            nc.tensor.transpose(tp[:], h_t[:, kb * P:(kb + 1) * P], ident[:])
                            o = eh.tile([P, P], bf16, tag="tpo")
                            nc.scalar.copy(o[:], tp[:])
                            nc.sync.dma_start(hT[l][kb * P:(kb + 1) * P, t * P:(t + 1) * P], o[:])

            rg = [list(range(NCORES))]

            def dummy_out():
                with tc.tile_pool(name="dummy", bufs=1) as dp:
                    z = dp.tile([P, D], f32)
                    nc.vector.memset(z[:], 0.0)
                    for t in range(NTILES):
                        nc.sync.dma_start(out_d[t * P:(t + 1) * P, :], z[:])

            steps = ["p1", "ag1", "e1", "p2", "ag2", "e2", None]
            upto = steps.index(stop_after) if stop_after else len(steps) - 1

            if upto >= 0:
                projections(0, xT[:], D)
            if upto >= 1:
                nc.gpsimd.collective_compute(
                    "AllGather", mybir.AluOpType.bypass, replica_groups=rg,
                    ins=[kvin[0][:].opt()], outs=[kvfull[0][:].opt()])
            if upto >= 2:
                edge_phase(0)
            if upto >= 3:
                projections(1, hT[0][:], HC)
            if upto >= 4:
                nc.gpsimd.collective_compute(
                    "AllGather", mybir.AluOpType.bypass, replica_groups=rg,
                    ins=[kvin[1][:].opt()], outs=[kvfull[1][:].opt()])
            if upto >= 5:
                edge_phase(1)
            if stop_after is not None:
                dummy_out()

            # final projection: out = h2 @ woutT + bout
            from contextlib import ExitStack
            with ExitStack() as _st:
              if stop_after is None:
                fw = _st.enter_context(tc.tile_pool(name="fw", bufs=1))
                fl = _st.enter_context(tc.tile_pool(name="fl", bufs=10))
                fo = _st.enter_context(tc.tile_pool(name="fo", bufs=2))
                fp = _st.enter_context(tc.tile_pool(name="fp", bufs=2, space="PSUM"))
                KB = HC // P
                wo_s = fw.tile([P, KB * D], bf16)
                nc.sync.dma_start(
                    wo_s[:].rearrange("p (kb n) -> p kb n", n=D),
                    woutT[:].rearrange("(kb p) n -> p kb n", p=P))
                for t in range(NTILES):
                    ps = fp.tile([P, D], f32, tag="po")
                    for kb in range(KB):
                        lt = fl.tile([P, P], bf16, tag="flhs")
                        nc.sync.dma_start(
                            lt[:], hT[1][kb * P:(kb + 1) * P, t * P:(t + 1) * P])
                        nc.tensor.matmul(
                            ps[:], lhsT=lt[:],
                            rhs=wo_s[:].rearrange("p (kb n) -> p kb n", n=D)[:, kb, :],
                            start=(kb == 0), stop=False)
                    nc.tensor.matmul(ps[:], lhsT=ones[:1, :], rhs=bout_s[:1, :],
                                     start=False, stop=True)
                    o = fo.tile([P, D], f32, tag="oo")
                    nc.vector.tensor_copy(o[:], ps[:])
                    nc.sync.dma_start(out_d[t * P:(t + 1) * P, :], o[:])

    _split_waits(nc)
    return nc


def _make_in_maps(inputs, srcidx, ninv, node_of_rank):
    x = np.ascontiguousarray(np.asarray(inputs["x"], np.float32))
    xsh = _shard_rows(x, node_of_rank)
    g = lambda n: np.ascontiguousarray(np.asarray(inputs[n], np.float32))
    common = {}
    for l in range(2):
        for nm in "qkvs":
            common[f"w{nm}{l}T"] = np.ascontiguousarray(g(f"{nm}{l}_w").T).astype(BF16)
            common[f"b{nm}{l}"] = g(f"{nm}{l}_b").reshape(1, HC).astype(BF16)
    common["woutT"] = np.ascontiguousarray(g("out_w").T).astype(BF16)
    common["bout"] = g("out_b").reshape(1, D).astype(BF16)
    in_maps = []
    for c in range(NCORES):
        m = dict(common)
        m["xT"] = np.ascontiguousarray(xsh[c].T).astype(BF16)
        m["srcidx"] = np.ascontiguousarray(srcidx[c])
        m["ninv"] = np.ascontiguousarray(ninv[c])
        in_maps.append(m)
    return in_maps


def kernel(**inputs):
    from concourse.bass_utils import run_bass_kernel_spmd
    srcidx, ninv, Dts, SUMD, coloff, node_of_rank = _prep(np.asarray(inputs["edge_index"]))
    nc = _build_nc(Dts, SUMD, coloff)
    in_maps = _make_in_maps(inputs, srcidx, ninv, node_of_rank)
    res = run_bass_kernel_spmd(nc, in_maps, core_ids=list(range(NCORES)))
    shards = np.stack([res.results[c]["out"] for c in range(NCORES)])
    full = np.empty((N, D), np.float32)
    r = np.arange(N)
    full[node_of_rank] = shards[r % NCORES, r // NCORES]
    return full


# revision 12
# speedup vs baseline: 2.0081x; 1.2236x over previous
"""GraphTransformer (2x PyG TransformerConv + out proj) on 8 trn2 NeuronCores.

Strategy (edge-parallel via dst-ownership):
- Host: sort nodes globally by (degree, id); rank r -> core r%8, local slot
  r//8. Every core's tile t covers the same global rank block => identical
  per-tile max degree D_t on all cores (SPMD-compatible). Edges grouped by
  dst; each dst's edges live entirely on its owner core as gather slots.
- Device, per layer: node-sharded bf16 projections on PE (q,k,v,skip),
  AllGather of fused k|v table [51200,1024] bf16; per 128-node tile:
  indirect-DMA gather of k|v rows per pow2-sized slot chunk, DVE dot via
  mult + pairwise fold + reduce, exp on scalar engine (softmax is shift
  invariant; logits are small, so no segment max needed). Invalid slots
  gather a zeroed padding row => contribute exp(0)=1 to the denom,
  corrected by adding the host-known -invalid_count+eps. Weighted sums
  fold pairwise in bf16, accumulate fp32, then skip-add + ReLU;
  PE-transpose of h feeds the next layer's stationary operand. Final
  projection -> per-core fp32 output shard; host un-permutes rows.
"""
import numpy as np
import ml_dtypes

N, E, D, H, C, HC = 50000, 400000, 384, 4, 128, 512
NCORES, P = 8, 128
NLOC = N // NCORES
NTILES = (NLOC + P - 1) // P
SHARD = NTILES * P
NPAD = SHARD * NCORES
SCHUNK = 8
INV_SQRT_C = 1.0 / np.sqrt(np.float32(C))
BF16 = ml_dtypes.bfloat16


def _pow2_chunks(d):
    out, s = [], SCHUNK
    while d > 0:
        while s > d:
            s //= 2
        out.append(s)
        d -= s
    return out


# ---------------------------------------------------------------- host prep
def _prep(edge_index):
    src = np.asarray(edge_index[0], dtype=np.int64)
    dst = np.asarray(edge_index[1], dtype=np.int64)
    deg = np.bincount(dst, minlength=N)
    node_of_rank = np.lexsort((np.arange(N), deg))
    rank_of_node = np.empty(N, np.int64)
    rank_of_node[node_of_rank] = np.arange(N)
    g_of_rank = (np.arange(N) % NCORES) * SHARD + (np.arange(N) // NCORES)
    g_of_node = np.empty(N, np.int64)
    g_of_node[node_of_rank] = g_of_rank

    deg_sorted = deg[node_of_rank]
    Dts = []
    for t in range(NTILES):
        blk = deg_sorted[t * P * NCORES:(t + 1) * P * NCORES]
        Dts.append(max(int(blk.max()) if len(blk) else 0, 1))
    SUMD = sum(Dts)
    coloff = np.cumsum([0] + Dts)[:-1]

    er = rank_of_node[dst]
    order = np.argsort(er, kind="stable")
    er_s = er[order]
    gsrc_s = g_of_node[src[order]]
    starts = np.searchsorted(er_s, np.arange(N))
    slot = np.arange(E) - starts[er_s]

    core_e = er_s % NCORES
    local_e = er_s // NCORES
    col_e = coloff[local_e // P] + slot
    p_e = local_e % P

    # invalid slots gather the (zeroed) padding row SHARD-1 of core 0
    srcidx = np.full((NCORES, P, SUMD), SHARD - 1, np.int32)
    valid = np.zeros((NCORES, P, SUMD), np.float32)
    srcidx[core_e, p_e, col_e] = gsrc_s.astype(np.int32)
    valid[core_e, p_e, col_e] = 1.0
    # per-(core, p, tile): -#invalid slots, added to the denominator
    ninv = np.zeros((NCORES, P, NTILES), np.float32)
    for t in range(NTILES):
        a, b = int(coloff[t]), int(coloff[t]) + Dts[t]
        ninv[:, :, t] = -(Dts[t] - valid[:, :, a:b].sum(axis=2))
    return srcidx, ninv, Dts, SUMD, coloff, node_of_rank


def _shard_rows(x, node_of_rank):
    D_in = x.shape[1]
    out = np.zeros((NCORES, SHARD, D_in), np.float32)
    r = np.arange(N)
    out[r % NCORES, r // NCORES] = x[node_of_rank]
    return out


# ---------------------------------------------------------------- wait fix
def _split_waits(nc):
    """walrus here rejects >1 sem-wait per instruction; split extras onto
    InstNoOp carriers inserted just before, same engine."""
    import concourse.mybir as mybir
    for fn in nc.m.functions:
        for bb in fn.blocks:
            out = []
            changed = False
            for ins in bb.instructions:
                si = ins.sync_info
                waits = list(si.on_wait) if si and si.on_wait else []
                if len(waits) > 1:
                    changed = True
                    for j, w in enumerate(waits[:-1]):
                        out.append(mybir.InstNoOp(
                            name=f"{ins.name}-wf{j}", opcode="NoOp",
                            engine=ins.engine,
                            sync_info=mybir.SyncInfo(on_wait=[w], on_update=[]),
                            text_hint="waitfix"))
                    si.on_wait = waits[-1:]
                out.append(ins)
            if changed:
                bb.instructions = out


# ---------------------------------------------------------------- bass build
def _build_nc(Dts, SUMD, coloff, stop_after=None, debug=False):
    import concourse.bass as bass
    import concourse.mybir as mybir
    import concourse.tile as tile
    from concourse.masks import make_identity
    f32 = mybir.dt.float32
    bf16 = mybir.dt.bfloat16

    nc = bass.Bass(num_devices=NCORES)
    xT = nc.dram_tensor("xT", [D, SHARD], bf16, kind="ExternalInput")
    srcidx_d = nc.dram_tensor("srcidx", [P, SUMD], mybir.dt.int32, kind="ExternalInput")
    ninv_d = nc.dram_tensor("ninv", [P, NTILES], f32, kind="ExternalInput")
    wT, bia = {}, {}
    for l, Din in ((0, D), (1, HC)):
        for nm in "qkvs":
            wT[nm, l] = nc.dram_tensor(f"w{nm}{l}T", [Din, HC], bf16, kind="ExternalInput")
            bia[nm, l] = nc.dram_tensor(f"b{nm}{l}", [1, HC], bf16, kind="ExternalInput")
    woutT = nc.dram_tensor("woutT", [HC, D], bf16, kind="ExternalInput")
    bout = nc.dram_tensor("bout", [1, D], bf16, kind="ExternalInput")
    out_d = nc.dram_tensor("out", [SHARD, D], f32, kind="ExternalOutput")
    dbg = {}
    if debug:
        dbg["kv0"] = nc.dram_tensor("dbg_kv0", [SHARD, 2 * HC], bf16, kind="ExternalOutput")
        dbg["qs0"] = nc.dram_tensor("dbg_qs0", [SHARD, 2 * HC], bf16, kind="ExternalOutput")
        dbg["kvfull0"] = nc.dram_tensor("dbg_kvfull0", [NPAD, 2 * HC], bf16, kind="ExternalOutput")
        dbg["h0T"] = nc.dram_tensor("dbg_h0T", [HC, SHARD], bf16, kind="ExternalOutput")
        dbg["den0"] = nc.dram_tensor("dbg_den0", [P, NTILES * H], mybir.dt.float32, kind="ExternalOutput")
        dbg["e0"] = nc.dram_tensor("dbg_e0", [P, NTILES * 24 * H], bf16, kind="ExternalOutput")
        dbg["msg0"] = nc.dram_tensor("dbg_msg0", [P, NTILES * HC], mybir.dt.float32, kind="ExternalOutput")

    DTMAX = max(Dts)
    chunks = []  # per tile: list of (abs_col, local_off, S)
    for t in range(NTILES):
        cs, off = [], 0
        for s in _pow2_chunks(Dts[t]):
            cs.append((int(coloff[t]) + off, off, s))
            off += s
        chunks.append(cs)

    with tile.TileContext(nc) as tc:
        with (
            tc.tile_pool(name="dram", bufs=1, space="DRAM") as dram,
            tc.tile_pool(name="const", bufs=1) as const,
        ):
            # persistent DRAM scratch
            qsd = [dram.tile([SHARD, 2 * HC], bf16, name=f"qs{l}d") for l in range(2)]
            kvin = [dram.tile([SHARD, 2 * HC], bf16, name=f"kv{l}in") for l in range(2)]
            kvfull = [dram.tile([NPAD, 2 * HC], bf16, name=f"kv{l}full", addr_space="Shared")
                      for l in range(2)]
            hT = [dram.tile([HC, SHARD], bf16, name=f"h{l}T") for l in range(2)]

            # constants in SBUF
            ident = const.tile([P, P], bf16)
            make_identity(nc, ident[:])
            ones = const.tile([1, P], bf16)
            nc.vector.memset(ones[:], 1.0)
            srcidx_s = const.tile([P, SUMD], mybir.dt.int32)
            nc.sync.dma_start(srcidx_s[:], srcidx_d[:])
            ninv_s = const.tile([P, NTILES], f32)
            nc.sync.dma_start(ninv_s[:], ninv_d[:])
            bias_s = {}
            for l in range(2):
                for nm in "qkvs":
                    bias_s[nm, l] = const.tile([1, HC], bf16, name=f"b{nm}{l}s")
                    nc.sync.dma_start(bias_s[nm, l][:], bia[nm, l][:])
            bout_s = const.tile([1, D], bf16)
            nc.sync.dma_start(bout_s[:], bout[:])

            def projections(l, lhsT_dram, Din):
                """q,k,v,s = lhsT.T @ W^T + b for this core's SHARD rows."""
                KB = Din // P
                with (
                    tc.tile_pool(name=f"wp{l}", bufs=1) as wp,
                    tc.tile_pool(name=f"lp{l}", bufs=3) as lp,
                    tc.tile_pool(name=f"op{l}", bufs=2) as op,
                    tc.tile_pool(name=f"pp{l}", bufs=2, space="PSUM") as pp,
                ):
                    w_s = {}
                    for nm in "qkvs":
                        w_s[nm] = wp.tile([P, KB * HC], bf16, name=f"w{nm}s")
                        nc.sync.dma_start(
                            w_s[nm][:].rearrange("p (kb n) -> p kb n", n=HC),
                            wT[nm, l][:].rearrange("(kb p) n -> p kb n", p=P))
                    # k,v first across all tiles so the AllGather can start as
                    # early as possible; q,s then overlap the collective.
                    for group in ("kv", "qs"):
                        for t in range(NTILES):
                            lt = lp.tile([P, KB * P], bf16, tag="lhs")
                            nc.sync.dma_start(
                                lt[:].rearrange("p (kb r) -> p kb r", r=P),
                                lhsT_dram[:, t * P:(t + 1) * P]
                                .rearrange("(kb p) r -> p kb r", p=P))
                            rows = slice(t * P, (t + 1) * P)
                            o = op.tile([P, 2 * HC], bf16, tag=f"o{group}")
                            for j, nm in enumerate(group):
                                ps = pp.tile([P, HC], f32, tag="ps")
                                for kb in range(KB):
                                    nc.tensor.matmul(
                                        ps[:], lhsT=lt[:, kb * P:(kb + 1) * P],
                                        rhs=w_s[nm][:].rearrange("p (kb n) -> p kb n", n=HC)[:, kb, :],
                                        start=(kb == 0), stop=False)
                                nc.tensor.matmul(
                                    ps[:], lhsT=ones[:1, :], rhs=bias_s[nm, l][:1, :],
                                    start=False, stop=True)
                                nc.scalar.copy(o[:, j * HC:(j + 1) * HC], ps[:])
                            if group == "kv":
                                nc.sync.dma_start(kvin[l][rows, :], o[:])
                            else:
                                nc.sync.dma_start(qsd[l][rows, :], o[:])
                    # zero the padding row so invalid slots gather k=v=0
                    zpad = op.tile([1, 2 * HC], bf16, tag="zpad")
                    nc.vector.memset(zpad[:], 0.0)
                    nc.sync.dma_start(kvin[l][SHARD - 1:SHARD, :], zpad[:])

            def edge_phase(l):
                with (
                    tc.tile_pool(name=f"ek{l}", bufs=3) as ek,
                    tc.tile_pool(name=f"eg{l}", bufs=2) as eg,
                    tc.tile_pool(name=f"eh{l}", bufs=2) as eh,
                    tc.tile_pool(name=f"et{l}", bufs=4, space="PSUM") as et,
                ):
                    for t in range(NTILES):
                        rows = slice(t * P, (t + 1) * P)
                        qs_t = eg.tile([P, 2 * HC], bf16, tag="qs")
                        nc.sync.dma_start(qs_t[:], qsd[l][rows, :])
                        ebuf = eh.tile([P, DTMAX * H], bf16, tag="ebuf")
                        msg = eh.tile([P, HC], f32, tag="msg")
                        for ci, (co, lo, S) in enumerate(chunks[t]):
                            kvg = ek.tile([P, SCHUNK * 2 * HC], bf16, tag="kvg")
                            for s in range(S):
                                nc.gpsimd.indirect_dma_start(
                                    out=kvg[:, s * 2 * HC:(s + 1) * 2 * HC],
                                    out_offset=None,
                                    in_=kvfull[l][:],
                                    in_offset=bass.IndirectOffsetOnAxis(
                                        ap=srcidx_s[:, co + s:co + s + 1], axis=0))
                            kv3 = kvg[:].rearrange("p (s kv) -> p s kv", kv=2 * HC)
                            prod = eg.tile([P, SCHUNK * HC], bf16, tag="prod")
                            nc.vector.tensor_tensor(
                                out=prod[:].rearrange("p (s n) -> p s n", n=HC)[:, :S],
                                in0=kv3[:, :S, 0:HC],
                                in1=qs_t[:, None, 0:HC].to_broadcast([P, S, HC]),
                                op=mybir.AluOpType.mult)
                            p5 = prod[:].rearrange(
                                "p (s h two c) -> p s h two c", h=H, two=2, c=C // 2)
                            foldc = eg.tile([P, SCHUNK * H * (C // 2)], bf16, tag="foldc")
                            nc.vector.tensor_tensor(
                                out=foldc[:].rearrange(
                                    "p (s h c) -> p s h c", h=H, c=C // 2)[:, :S],
                                in0=p5[:, :S, :, 0, :], in1=p5[:, :S, :, 1, :],
                                op=mybir.AluOpType.add)
                            alpha = eh.tile([P, SCHUNK * H], f32, tag="alpha")
                            nc.vector.tensor_reduce(
                                alpha[:, :S * H],
                                foldc[:, :S * H * (C // 2)].rearrange(
                                    "p (sh c) -> p sh c", c=C // 2),
                                axis=mybir.AxisListType.X, op=mybir.AluOpType.add)
                            nc.scalar.activation(
                                ebuf[:, lo * H:(lo + S) * H], alpha[:, :S * H],
                                mybir.ActivationFunctionType.Exp, scale=float(INV_SQRT_C))
                            ev = eg.tile([P, SCHUNK * HC], bf16, tag="prod")
                            nc.vector.tensor_tensor(
                                out=ev[:].rearrange("p (s h c) -> p s h c", h=H, c=C)[:, :S],
                                in0=kv3[:, :S, HC:2 * HC].rearrange("p s (h c) -> p s h c", c=C),
                                in1=ebuf[:, lo * H:(lo + S) * H]
                                    .rearrange("p (s h) -> p s h", h=H)
                                    [:, :, :, None].to_broadcast([P, S, H, C]),
                                op=mybir.AluOpType.mult)
                            n = S
                            while n > 1:
                                half = n // 2
                                nc.vector.tensor_tensor(
                                    out=ev[:, :half * HC].rearrange("p (s n) -> p s n", n=HC),
                                    in0=ev[:, :half * HC].rearrange("p (s n) -> p s n", n=HC),
                                    in1=ev[:, half * HC:n * HC].rearrange("p (s n) -> p s n", n=HC),
                                    op=mybir.AluOpType.add)
                                n = half
                            if ci == 0:
                                nc.scalar.copy(msg[:], ev[:, :HC])
                            else:
                                nc.vector.tensor_add(msg[:], msg[:], ev[:, :HC])
                        den = eh.tile([P, H], f32, tag="den")
                        nc.vector.tensor_reduce(
                            den[:], ebuf[:, :Dts[t] * H].rearrange("p (s h) -> p h s", h=H),
                            axis=mybir.AxisListType.X, op=mybir.AluOpType.add)
                        # add -#invalid (each contributed exp(0)=1); clamp off 0
                        nc.vector.tensor_add(
                            den[:], den[:], ninv_s[:, t:t + 1].to_broadcast([P, H]))
                        nc.vector.tensor_scalar_max(den[:], den[:], 1e-16)
                        if debug and l == 0:
                            nc.sync.dma_start(dbg["den0"][:, t * H:(t + 1) * H], den[:])
                            nc.sync.dma_start(
                                dbg["e0"][:, t * 24 * H:t * 24 * H + Dts[t] * H],
                                ebuf[:, :Dts[t] * H])
                            nc.sync.dma_start(dbg["msg0"][:, t * HC:(t + 1) * HC], msg[:])
                        rden = eh.tile([P, H], f32, tag="rden")
                        nc.vector.reciprocal(rden[:], den[:])
                        h_t = eh.tile([P, HC], bf16, tag="h")
                        nc.vector.tensor_tensor(
                            out=h_t[:].rearrange("p (h c) -> p h c", c=C),
                            in0=msg[:].rearrange("p (h c) -> p h c", c=C),
                            in1=rden[:, :, None].to_broadcast([P, H, C]),
                            op=mybir.AluOpType.mult)
                        nc.vector.tensor_add(h_t[:], h_t[:], qs_t[:, HC:2 * HC])
                        nc.scalar.activation(h_t[:], h_t[:], mybir.ActivationFunctionType.Relu)
                        o4 = eh.tile([P, (HC // P) * P], bf16, tag="tpo")
                        for kb in range(HC // P):
                            tp = et.tile([P, P], bf16, tag="tp")
                            nc.tensor.transpose(tp[:], h_t[:, kb * P:(kb + 1) * P], ident[:])
                            nc.scalar.copy(o4[:, kb * P:(kb + 1) * P], tp[:])
                        nc.sync.dma_start(
                            hT[l][:].rearrange("(kb p) r -> p kb r", p=P)
                            [:, :, t * P:(t + 1) * P],
                            o4[:].rearrange("p (kb r) -> p kb r", r=P))

            rg = [list(range(NCORES))]

            def dummy_out():
                with tc.tile_pool(name="dummy", bufs=1) as dp:
                    z = dp.tile([P, D], f32)
                    nc.vector.memset(z[:], 0.0)
                    for t in range(NTILES):
                        nc.sync.dma_start(out_d[t * P:(t + 1) * P, :], z[:])

            steps = ["p1", "ag1", "e1", "p2", "ag2", "e2", None]
            upto = steps.index(stop_after) if stop_after else len(steps) - 1

            if upto >= 0:
                projections(0, xT[:], D)
            if upto >= 1:
                nc.gpsimd.collective_compute(
                    "AllGather", mybir.AluOpType.bypass, replica_groups=rg,
                    ins=[kvin[0][:].opt()], outs=[kvfull[0][:].opt()])
            if upto >= 2:
                edge_phase(0)
            if upto >= 3:
                projections(1, hT[0][:], HC)
            if upto >= 4:
                nc.gpsimd.collective_compute(
                    "AllGather", mybir.AluOpType.bypass, replica_groups=rg,
                    ins=[kvin[1][:].opt()], outs=[kvfull[1][:].opt()])
            if upto >= 5:
                edge_phase(1)
            if stop_after is not None:
                dummy_out()
            if debug:
                nc.sync.dma_start(dbg["kv0"][:], kvin[0][:])
                nc.sync.dma_start(dbg["qs0"][:], qsd[0][:])
                nc.sync.dma_start(dbg["kvfull0"][:], kvfull[0][:])
                nc.sync.dma_start(dbg["h0T"][:], hT[0][:])

            # final projection: out = h2 @ woutT + bout
            from contextlib import ExitStack
            with ExitStack() as _st:
              if stop_after is None:
                fw = _st.enter_context(tc.tile_pool(name="fw", bufs=1))
                fl = _st.enter_context(tc.tile_pool(name="fl", bufs=3))
                fo = _st.enter_context(tc.tile_pool(name="fo", bufs=2))
                fp = _st.enter_context(tc.tile_pool(name="fp", bufs=2, space="PSUM"))
                KB = HC // P
                wo_s = fw.tile([P, KB * D], bf16)
                nc.sync.dma_start(
                    wo_s[:].rearrange("p (kb n) -> p kb n", n=D),
                    woutT[:].rearrange("(kb p) n -> p kb n", p=P))
                for t in range(NTILES):
                    lt = fl.tile([P, KB * P], bf16, tag="flhs")
                    nc.sync.dma_start(
                        lt[:].rearrange("p (kb r) -> p kb r", r=P),
                        hT[1][:, t * P:(t + 1) * P].rearrange("(kb p) r -> p kb r", p=P))
                    ps = fp.tile([P, D], f32, tag="po")
                    for kb in range(KB):
                        nc.tensor.matmul(
                            ps[:], lhsT=lt[:, kb * P:(kb + 1) * P],
                            rhs=wo_s[:].rearrange("p (kb n) -> p kb n", n=D)[:, kb, :],
                            start=(kb == 0), stop=False)
                    nc.tensor.matmul(ps[:], lhsT=ones[:1, :], rhs=bout_s[:1, :],
                                     start=False, stop=True)
                    o = fo.tile([P, D], f32, tag="oo")
                    nc.scalar.copy(o[:], ps[:])
                    nc.sync.dma_start(out_d[t * P:(t + 1) * P, :], o[:])

    _split_waits(nc)
    return nc


def _make_in_maps(inputs, srcidx, ninv, node_of_rank):
    x = np.ascontiguousarray(np.asarray(inputs["x"], np.float32))
    xsh = _shard_rows(x, node_of_rank)
    g = lambda n: np.ascontiguousarray(np.asarray(inputs[n], np.float32))
    common = {}
    for l in range(2):
        for nm in "qkvs":
            common[f"w{nm}{l}T"] = np.ascontiguousarray(g(f"{nm}{l}_w").T).astype(BF16)
            common[f"b{nm}{l}"] = g(f"{nm}{l}_b").reshape(1, HC).astype(BF16)
    common["woutT"] = np.ascontiguousarray(g("out_w").T).astype(BF16)
    common["bout"] = g("out_b").reshape(1, D).astype(BF16)
    in_maps = []
    for c in range(NCORES):
        m = dict(common)
        m["xT"] = np.ascontiguousarray(xsh[c].T).astype(BF16)
        m["srcidx"] = np.ascontiguousarray(srcidx[c])
        m["ninv"] = np.ascontiguousarray(ninv[c])
        in_maps.append(m)
    return in_maps


def kernel(**inputs):
    from concourse.bass_utils import run_bass_kernel_spmd
    srcidx, ninv, Dts, SUMD, coloff, node_of_rank = _prep(np.asarray(inputs["edge_index"]))
    nc = _build_nc(Dts, SUMD, coloff)
    in_maps = _make_in_maps(inputs, srcidx, ninv, node_of_rank)
    res = run_bass_kernel_spmd(nc, in_maps, core_ids=list(range(NCORES)))
    shards = np.stack([res.results[c]["out"] for c in range(NCORES)])
    full = np.empty((N, D), np.float32)
    r = np.arange(N)
    full[node_of_rank] = shards[r % NCORES, r // NCORES]
    return full
